# revision 10
# baseline (speedup 1.0000x reference)
"""Trainium2 Bass kernel for nn_CGPBlock (attention block with 1x1-conv QKV).

Reference computation (per batch b):
    q = Wq @ pose + bq; k = Wk @ id + bk; v = Wv @ pose + bv     # [C, L]
    energy[i, j] = sum_c q[c, i] k[c, j]                          # [L, L]
    attn = softmax_j(energy)
    va[c, i] = sum_j v[c, j] attn[i, j]
    out = pose + gamma * va

Sharding: data-parallel over batch, B=8 batches -> 8 NeuronCores (SPMD, no
collectives). Per core: C=128 fills the partition dim, L=4096.

Device algorithm (per core, matmuls bf16 with fp32 PSUM accumulate):
  - For each i-chunk (1024), loop j-tiles (128): eT[j,i] = k_jt.T @ q_chunk
    (PSUM), then softmax numerator p = exp(eT) WITHOUT max subtraction
    (|E| < 40 in practice, fp32/bf16 safe), then va += vt_jt.T @ p (PSUM
    accumulate) and Z (softmax denominators) via ones.T @ (presummed p).
  - exp is split across TWO engines to halve the softmax-stage cost:
    ACT does real exp; DVE computes a Schraudolph approximation in one
    tensor_scalar op: int16(E*(128/ln2) + 16248.75) IS the bf16 bit
    pattern of exp(E) with a linear-mantissa approx (~1.8% rms error,
    cancels in va/Z normalization; tolerance is 2e-2).
  - p j-tile PAIRS share one [C, 2048] tile so Z presums run as wide DVE
    tensor_tensor adds (3x 2048-wide + 1 fold per 8 j-tiles), then one
    M=1 matmul per 8 j-tiles accumulates Z in PSUM.
  - Startup: PE clock-gate warmed by garbage matmuls on an uninitialized
    SBUF tile (no DMA/DVE dependency -> starts ~0.3us, HAM warm by ~4us).
    Inputs spread across 4 DMA queues (sync/scalar HWDGE, gpsimd SWDGE)
    so the first convs start ~10us and energy ~15us (vs 25us serial).
    vt (v transposed) for chunk 0 is produced as two half-transposes on
    the two HWDGE xbars in parallel to beat the first va matmul.
  - ch1-3 v/q convs + drains are interleaved INTO chunk 0's attention
    stream (PSUM tiles recycled from the energy pool) so all k convs can
    run up front (energy needs every k chunk) without extra PSUM.
  - Tail: last chunk's Z ends in 4 direct M=1 matmuls (skew decayed to 2)
    and the normalize runs once at full width with a PE-broadcast 1/Z,
    output split across 4 DMA queues.
"""

import numpy as np
import ml_dtypes

import concourse.bacc as bacc
import concourse.tile as tile
from concourse import mybir
from concourse.bass_utils import run_bass_kernel_spmd

F32 = mybir.dt.float32
BF16 = mybir.dt.bfloat16
I16 = mybir.dt.int16
AF = mybir.ActivationFunctionType
ALU = mybir.AluOpType

B, C, L = 8, 128, 4096
CHUNK = 1024                # i-chunk width
NCH = L // CHUNK            # 4 chunks
NJT = L // 128              # 32 j-tiles per chunk
JPC = CHUNK // 128          # j-tiles per k/v chunk tile
SKEW = 8                    # software pipeline depth (PE runs ahead of exp)
WARM_MMS = 10               # garbage matmuls covering the input-DMA window

SCHRAUD_A = 128.0 / float(np.log(2.0))   # 184.664965...
SCHRAUD_B = 16248.75                     # round-to-nearest calibrated

# j-tiles whose exp runs on DVE (Schraudolph) instead of ACT, per chunk.
# ch0 front-loads DVE while va hasn't started (PE outruns a single ACT);
# last chunk keeps the final 8 tiles on ACT (direct-Z tail).
DVE_JTS = {
    0: {4, 6, 8, 10, 12, 17, 22, 27},
    1: {2, 7, 12, 17, 22, 27, 30},
    2: {2, 7, 12, 17, 22, 27, 30},
    3: {2, 7, 12, 17, 20, 23},
}

_CACHE = {}


def _build():
    nc = bacc.Bacc("TRN2", target_bir_lowering=False, debug=False, num_devices=B)

    pose_d = nc.dram_tensor("pose", [C, L], F32, kind="ExternalInput").ap()
    posebf_d = nc.dram_tensor("posebf", [C, L], BF16, kind="ExternalInput").ap()
    idbf_d = nc.dram_tensor("idbf", [C, L], BF16, kind="ExternalInput").ap()
    wt_d = nc.dram_tensor("wt", [C, 3 * C], BF16, kind="ExternalInput").ap()
    bq_d = nc.dram_tensor("bq", [C, 1], F32, kind="ExternalInput").ap()
    bk_d = nc.dram_tensor("bk", [C, 1], F32, kind="ExternalInput").ap()
    bfin_d = nc.dram_tensor("bfin", [C, 1], F32, kind="ExternalInput").ap()
    gam_d = nc.dram_tensor("gam", [C, 1], F32, kind="ExternalInput").ap()
    out_d = nc.dram_tensor("out", [C, L], F32, kind="ExternalOutput").ap()

    with tile.TileContext(nc) as tc:
        with tc.tile_pool(name="res", bufs=1) as res:
            # ---- input DMAs spread over queues ----
            # sync:   wt, idbf0, idbf1, bq, bk   | vt transposes, out halves
            # scalar: posebf0, idbf2, idbf3, bfin, gam | vt transposes, outs
            # gpsimd: posebf1..3, pose0..3
            wt_sb = res.tile([C, 3 * C], BF16)
            nc.sync.dma_start(wt_sb, wt_d)

            def chunk_tiles(prefix, dtype):
                return [res.tile([C, CHUNK], dtype, name=f"{prefix}{i}")
                        for i in range(NCH)]

            pose_t = chunk_tiles("pose", F32)
            posebf_t = chunk_tiles("posebf", BF16)
            idbf_t = chunk_tiles("idbf", BF16)
            q_t = chunk_tiles("q", BF16)
            k_t = chunk_tiles("k", BF16)
            v_t = chunk_tiles("v", BF16)
            vt_t = chunk_tiles("vt", BF16)   # [j (partition), jt*128 + c]

            def dslice(ch):
                return slice(ch * CHUNK, (ch + 1) * CHUNK)

            nc.sync.dma_start(idbf_t[0], idbf_d[:, dslice(0)])
            nc.sync.dma_start(idbf_t[1], idbf_d[:, dslice(1)])
            bq_sb = res.tile([C, 1], F32)
            bk_sb = res.tile([C, 1], F32)
            nc.sync.dma_start(bq_sb, bq_d)
            nc.sync.dma_start(bk_sb, bk_d)

            nc.scalar.dma_start(posebf_t[0], posebf_d[:, dslice(0)])
            nc.scalar.dma_start(idbf_t[2], idbf_d[:, dslice(2)])
            nc.scalar.dma_start(idbf_t[3], idbf_d[:, dslice(3)])
            bfin_sb = res.tile([C, 1], F32)
            gam_sb = res.tile([C, 1], F32)
            nc.scalar.dma_start(bfin_sb, bfin_d)
            nc.scalar.dma_start(gam_sb, gam_d)

            for ch in range(1, NCH):
                nc.gpsimd.dma_start(posebf_t[ch], posebf_d[:, dslice(ch)])
            for ch in range(NCH):
                nc.gpsimd.dma_start(pose_t[ch], pose_d[:, dslice(ch)])

            # bf16 1.0 const: memset at Bass init (before user code) — the
            # Z-matmul ones vector and a dependency-free warmup operand.
            ones_sb = nc.const_aps.tensor(1.0, (C, 1), BF16)
            onesr_sb = res.tile([1, C], F32)
            nc.vector.memset(onesr_sb, 1.0)

            wqT = wt_sb[:, 0:C]
            wkT = wt_sb[:, C:2 * C]
            wvT = wt_sb[:, 2 * C:3 * C]

            def conv_mms(psum, w, src):
                for h in range(CHUNK // 512):
                    hs = slice(h * 512, (h + 1) * 512)
                    nc.tensor.matmul(psum[:, hs], lhsT=w, rhs=src[:, hs],
                                     start=True, stop=True)

            # ---- warmup + head convs share one PSUM pool (separate tags)
            # so the convs don't wait for a warmup-pool close. Warm MMs:
            # stage 1 on the const AP (zero deps — starts the instant the
            # PE queue opens, flips HAM to 8/8), stage 2 wider, covering
            # until the input DMAs land. Convs v0,q0,k0..k3 follow (energy
            # ch0 needs every k chunk).
            warm_sb = res.tile([C, 384], BF16)
            nc.vector.memset(warm_sb, 0.0)
            with tc.tile_pool(name="conv_ps", bufs=3, space="PSUM") as conv_ps:
                wp1 = conv_ps.tile([1, 1], F32, tag="w1", bufs=1)
                for _ in range(60):
                    nc.tensor.matmul(wp1, lhsT=ones_sb, rhs=ones_sb,
                                     start=True, stop=True)
                wp2 = conv_ps.tile([1, 384], F32, tag="w2", bufs=1)
                for _ in range(WARM_MMS):
                    nc.tensor.matmul(wp2, lhsT=ones_sb, rhs=warm_sb,
                                     start=True, stop=True)
                vp0 = conv_ps.tile([C, CHUNK], F32, tag="cv", name="vp0")
                conv_mms(vp0, wvT, posebf_t[0])
                # v0 drained in halves so both HWDGE xbars transpose in parallel
                nc.scalar.copy(v_t[0][:, 0:512], vp0[:, 0:512])
                nc.scalar.copy(v_t[0][:, 512:1024], vp0[:, 512:1024])
                nc.sync.dma_start_transpose(
                    vt_t[0][:, 0:512].rearrange("p (t c) -> p t c", c=C),
                    v_t[0][:, 0:512])
                nc.scalar.dma_start_transpose(
                    vt_t[0][:, 512:1024].rearrange("p (t c) -> p t c", c=C),
                    v_t[0][:, 512:1024])
                qp0 = conv_ps.tile([C, CHUNK], F32, tag="cv", name="qp0")
                conv_mms(qp0, wqT, posebf_t[0])
                nc.vector.tensor_scalar_add(q_t[0], qp0, bq_sb)
                for ch in range(NCH):
                    kp = conv_ps.tile([C, CHUNK], F32, tag="cv", name=f"kp{ch}")
                    conv_mms(kp, wkT, idbf_t[ch])
                    if ch < 2:
                        nc.scalar.activation(k_t[ch], kp, AF.Identity,
                                             bias=bk_sb)
                    else:
                        nc.vector.tensor_scalar_add(k_t[ch], kp, bk_sb)

            # ---- attention ----
            with (
                tc.tile_pool(name="et_ps", bufs=2, space="PSUM") as et_ps,
                tc.tile_pool(name="va_ps", bufs=1, space="PSUM") as va_ps,
                tc.tile_pool(name="z_ps", bufs=1, space="PSUM") as z_ps,
                tc.tile_pool(name="pt_sb", bufs=6) as pt_pool,
                tc.tile_pool(name="qs_sb", bufs=2) as qs_pool,
                tc.tile_pool(name="nrm", bufs=2) as nrm,
                tc.tile_pool(name="outb", bufs=2) as outb,
                tc.tile_pool(name="dramp", bufs=2, space="DRAM") as dramp,
            ):
                for ch in range(NCH):
                    i0 = ch * CHUNK
                    isl = slice(i0, i0 + CHUNK)
                    last = ch == NCH - 1
                    dve_set = DVE_JTS[ch]
                    va = va_ps.tile([C, CHUNK], F32)
                    z = z_ps.tile([1, CHUNK], F32)
                    pairs = {}      # pair idx -> [C, 2*CHUNK] tile
                    ssums = {}      # group idx -> [C, CHUNK] presum (8 jt)
                    asums = {}      # group idx -> [C, 2*CHUNK] A-partial

                    def skew_at(j):
                        if not last:
                            return SKEW
                        return max(2, min(SKEW, NJT + 2 - j))

                    lag_ptr = 0
                    n_extra = 2 if last else SKEW
                    for jt in range(NJT + n_extra):
                        if jt < NJT:
                            ksl = slice((jt % JPC) * 128, (jt % JPC + 1) * 128)
                            et = et_ps.tile([C, CHUNK], F32, tag="et")
                            for h in range(CHUNK // 512):
                                hs = slice(h * 512, (h + 1) * 512)
                                nc.tensor.matmul(
                                    et[:, hs], lhsT=k_t[jt // JPC][:, ksl],
                                    rhs=q_t[ch][:, hs],
                                    start=True, stop=True)
                            pr = jt // 2
                            if jt % 2 == 0:
                                pairs[pr] = pt_pool.tile([C, 2 * CHUNK], BF16,
                                                         name="pt")
                            half = pairs[pr][:, (jt % 2) * CHUNK:
                                             (jt % 2 + 1) * CHUNK]
                            if jt in dve_set:
                                nc.vector.tensor_scalar(
                                    out=half.bitcast(I16), in0=et,
                                    scalar1=SCHRAUD_A, scalar2=SCHRAUD_B,
                                    op0=ALU.mult, op1=ALU.add)
                            else:
                                nc.scalar.activation(half, et, AF.Exp)

                            # ch1-3 v/q convs ride chunk 0's stream
                            if ch == 0 and jt in (6, 16, 26):
                                cc = {6: 1, 16: 2, 26: 3}[jt]
                                vp = et_ps.tile([C, CHUNK], F32, tag="et",
                                                name=f"vp{cc}")
                                conv_mms(vp, wvT, posebf_t[cc])
                                nc.scalar.copy(v_t[cc], vp)
                                eng = nc.sync if cc != 2 else nc.scalar
                                eng.dma_start_transpose(
                                    vt_t[cc].rearrange("p (t c) -> p t c", c=C),
                                    v_t[cc])
                                qp = et_ps.tile([C, CHUNK], F32, tag="et",
                                                name=f"qp{cc}")
                                conv_mms(qp, wqT, posebf_t[cc])
                                nc.vector.tensor_scalar_add(q_t[cc], qp, bq_sb)

                            # Z presums on pair tiles (skip tail tiles of
                            # the last chunk: those go direct)
                            g = jt // 8
                            if jt % 8 == 3 and not (last and g == 3):
                                a = qs_pool.tile([C, 2 * CHUNK], BF16, tag="a")
                                nc.vector.tensor_add(a, pairs[4 * g],
                                                     pairs[4 * g + 1])
                                asums[g] = a
                            if jt % 8 == 7 and not (last and g == 3):
                                b_ = qs_pool.tile([C, 2 * CHUNK], BF16, tag="b")
                                nc.vector.tensor_add(b_, pairs[4 * g + 2],
                                                     pairs[4 * g + 3])
                                cs = qs_pool.tile([C, 2 * CHUNK], BF16, tag="c")
                                nc.vector.tensor_add(cs, asums.pop(g), b_)
                                s = qs_pool.tile([C, CHUNK], BF16, tag="s",
                                                 bufs=3)
                                nc.vector.tensor_add(s, cs[:, 0:CHUNK],
                                                     cs[:, CHUNK:2 * CHUNK])
                                ssums[g] = s
                            if last and jt == 27:
                                # partial group 3: jts 24-27 presummed
                                a = qs_pool.tile([C, 2 * CHUNK], BF16, tag="a")
                                nc.vector.tensor_add(a, pairs[12], pairs[13])
                                s = qs_pool.tile([C, CHUNK], BF16, tag="s",
                                                 bufs=3)
                                nc.vector.tensor_add(s, a[:, 0:CHUNK],
                                                     a[:, CHUNK:2 * CHUNK])
                                ssums[3] = s
                            # pose' = pose + gamma*bv, on ACT (free affine)
                            if jt == 29:
                                nc.scalar.activation(pose_t[ch], pose_t[ch],
                                                     AF.Identity, bias=bfin_sb)

                        while lag_ptr <= min(jt - skew_at(jt), NJT - 1):
                            lag = lag_ptr
                            lag_ptr += 1
                            vsl = slice((lag % JPC) * 128,
                                        (lag % JPC + 1) * 128)
                            phalf = pairs[lag // 2][:, (lag % 2) * CHUNK:
                                                    (lag % 2 + 1) * CHUNK]
                            for h in range(CHUNK // 512):
                                hs = slice(h * 512, (h + 1) * 512)
                                nc.tensor.matmul(
                                    va[:, hs], lhsT=vt_t[lag // JPC][:, vsl],
                                    rhs=phalf[:, hs],
                                    start=(lag == 0),
                                    stop=(lag == NJT - 1))
                            lg = lag // 8
                            if last and lag >= 28:
                                # tail: direct M=1 Z matmuls, no DVE chain
                                for h in range(CHUNK // 512):
                                    hs = slice(h * 512, (h + 1) * 512)
                                    nc.tensor.matmul(z[0:1, hs], lhsT=ones_sb,
                                                     rhs=phalf[:, hs],
                                                     start=False,
                                                     stop=(lag == NJT - 1))
                            elif lag % 8 == 7 or (last and lag == 27):
                                s = ssums.pop(lg)
                                for h in range(CHUNK // 512):
                                    hs = slice(h * 512, (h + 1) * 512)
                                    nc.tensor.matmul(
                                        z[0:1, hs], lhsT=ones_sb,
                                        rhs=s[:, hs], start=(lg == 0),
                                        stop=(not last and lg == 3))

                    # free the va PSUM bank quickly, then normalize from SBUF
                    va_sb = nrm.tile([C, CHUNK], F32, tag="va_sb")
                    nc.vector.tensor_copy(va_sb, va)
                    rz = nrm.tile([1, CHUNK], F32, tag="rz")
                    if not last:
                        nc.vector.reciprocal_approx_fast(rz, z)
                        # broadcast 1/Z across partitions via DRAM round-trip
                        # on the idle gpsimd queue (hidden under next chunk)
                        zd = dramp.tile([1, CHUNK], F32)
                        nc.gpsimd.dma_start(zd, rz)
                        rzb = nrm.tile([C, CHUNK], F32, tag="rzb")
                        nc.gpsimd.dma_start(rzb, zd.to_broadcast([C, CHUNK]))
                        t = nrm.tile([C, CHUNK], F32, tag="t")
                        nc.vector.tensor_mul(t, va_sb, rzb)
                        o = outb.tile([C, CHUNK], F32)
                        nc.vector.scalar_tensor_tensor(
                            o, in0=t, scalar=gam_sb, in1=pose_t[ch],
                            op0=ALU.mult, op1=ALU.add)
                        nc.sync.dma_start(out_d[:, i0:i0 + 512], o[:, 0:512])
                        nc.scalar.dma_start(out_d[:, i0 + 512:i0 + CHUNK],
                                            o[:, 512:1024])
                    else:
                        # tail: PE broadcasts 1/Z (idle now); normalize in
                        # two pipelined 512-halves so the first out-DMA
                        # overlaps the second half's compute
                        rzb = et_ps.tile([C, CHUNK], F32, tag="et",
                                         name="rzb_ps")
                        t = nrm.tile([C, CHUNK], F32, tag="t")
                        o = outb.tile([C, CHUNK], F32)
                        for h in range(CHUNK // 512):
                            hs = slice(h * 512, (h + 1) * 512)
                            ihs = slice(i0 + h * 512, i0 + (h + 1) * 512)
                            nc.vector.reciprocal_approx_fast(rz[0:1, hs],
                                                             z[0:1, hs])
                            nc.tensor.matmul(rzb[:, hs], lhsT=onesr_sb,
                                             rhs=rz[0:1, hs],
                                             start=True, stop=True)
                            nc.vector.tensor_mul(t[:, hs], va_sb[:, hs],
                                                 rzb[:, hs])
                            nc.vector.scalar_tensor_tensor(
                                o[:, hs], in0=t[:, hs], scalar=gam_sb,
                                in1=pose_t[ch][:, hs],
                                op0=ALU.mult, op1=ALU.add)
                            eng = nc.sync if h == 0 else nc.scalar
                            eng.dma_start(out_d[:, ihs], o[:, hs])

    nc.compile()
    return nc


def _get_nc():
    if "nc" not in _CACHE:
        _CACHE["nc"] = _build()
    return _CACHE["nc"]


def kernel(pose_f, id_f, Wq, bq, Wk, bk, Wv, bv, gamma, **run_kwargs):
    pose_f = np.asarray(pose_f, dtype=np.float32)
    id_f = np.asarray(id_f, dtype=np.float32)
    Wq = np.asarray(Wq, dtype=np.float32)
    Wk = np.asarray(Wk, dtype=np.float32)
    Wv = np.asarray(Wv, dtype=np.float32)
    bq = np.asarray(bq, dtype=np.float32)
    bk = np.asarray(bk, dtype=np.float32)
    bv = np.asarray(bv, dtype=np.float32)
    g = float(np.asarray(gamma, dtype=np.float32).reshape(-1)[0])

    bf = ml_dtypes.bfloat16
    wt = np.concatenate([Wq.T, Wk.T, Wv.T], axis=1).astype(bf)  # [C_in, 3C]
    posebf = pose_f.astype(bf)
    idbf = id_f.astype(bf)
    bq_c = np.ascontiguousarray(bq.reshape(C, 1))
    bk_c = np.ascontiguousarray(bk.reshape(C, 1))
    bfin = np.ascontiguousarray((g * bv).reshape(C, 1).astype(np.float32))
    gam = np.full((C, 1), g, dtype=np.float32)

    in_maps = []
    for b in range(B):
        in_maps.append({
            "pose": pose_f[b],
            "posebf": posebf[b],
            "idbf": idbf[b],
            "wt": wt,
            "bq": bq_c,
            "bk": bk_c,
            "bfin": bfin,
            "gam": gam,
        })

    nc = _get_nc()
    res = run_bass_kernel_spmd(nc, in_maps, core_ids=list(range(B)), **run_kwargs)
    out = np.stack([res.results[b]["out"] for b in range(B)], axis=0)
    if run_kwargs:
        _CACHE["last_result"] = res
    return out


# revision 14
# speedup vs baseline: 1.0249x; 1.0249x over previous
"""Trainium2 Bass kernel for nn_CGPBlock (attention block with 1x1-conv QKV).

Reference computation (per batch b):
    q = Wq @ pose + bq; k = Wk @ id + bk; v = Wv @ pose + bv     # [C, L]
    energy[i, j] = sum_c q[c, i] k[c, j]                          # [L, L]
    attn = softmax_j(energy)
    va[c, i] = sum_j v[c, j] attn[i, j]
    out = pose + gamma * va

Sharding: data-parallel over batch, B=8 batches -> 8 NeuronCores (SPMD, no
collectives). Per core: C=128 fills the partition dim, L=4096.

Device algorithm (per core, matmuls bf16 with fp32 PSUM accumulate):
  - For each i-chunk (1024), loop j-tiles (128): eT[j,i] = k_jt.T @ q_chunk
    (PSUM), then softmax numerator p = exp(eT) WITHOUT max subtraction
    (|E| < 40 in practice, fp32/bf16 safe), then va += vt_jt.T @ p (PSUM
    accumulate) and Z (softmax denominators) via ones.T @ (presummed p).
  - exp is split across TWO engines to halve the softmax-stage cost:
    ACT does real exp; DVE computes a Schraudolph approximation in one
    tensor_scalar op: int16(E*(128/ln2) + 16248.75) IS the bf16 bit
    pattern of exp(E) with a linear-mantissa approx (~1.8% rms error,
    cancels in va/Z normalization; tolerance is 2e-2).
  - p j-tile PAIRS share one [C, 2048] tile so Z presums run as wide DVE
    tensor_tensor adds (3x 2048-wide + 1 fold per 8 j-tiles), then one
    M=1 matmul per 8 j-tiles accumulates Z in PSUM.
  - Startup: PE clock-gate warmed by garbage matmuls on an uninitialized
    SBUF tile (no DMA/DVE dependency -> starts ~0.3us, HAM warm by ~4us).
    Inputs spread across 4 DMA queues (sync/scalar HWDGE, gpsimd SWDGE)
    so the first convs start ~10us and energy ~15us (vs 25us serial).
    vt (v transposed) for chunk 0 is produced as two half-transposes on
    the two HWDGE xbars in parallel to beat the first va matmul.
  - ch1-3 v/q convs + drains are interleaved INTO chunk 0's attention
    stream (PSUM tiles recycled from the energy pool) so all k convs can
    run up front (energy needs every k chunk) without extra PSUM.
  - Tail: last chunk's Z ends in 4 direct M=1 matmuls (skew decayed to 2)
    and the normalize runs once at full width with a PE-broadcast 1/Z,
    output split across 4 DMA queues.
"""

import numpy as np
import ml_dtypes

import concourse.bacc as bacc
import concourse.tile as tile
from concourse import mybir
from concourse.bass_utils import run_bass_kernel_spmd

F32 = mybir.dt.float32
BF16 = mybir.dt.bfloat16
I16 = mybir.dt.int16
AF = mybir.ActivationFunctionType
ALU = mybir.AluOpType

B, C, L = 8, 128, 4096
CHUNK = 1024                # i-chunk width
NCH = L // CHUNK            # 4 chunks
NJT = L // 128              # 32 j-tiles per chunk
JPC = CHUNK // 128          # j-tiles per k/v chunk tile
SKEW = 8                    # software pipeline depth (PE runs ahead of exp)
WARM_MMS = 12               # garbage matmuls covering the input-DMA window

SCHRAUD_A = 128.0 / float(np.log(2.0))   # 184.664965...
SCHRAUD_B = 16248.75                     # round-to-nearest calibrated

# j-tiles whose exp runs on DVE (Schraudolph) instead of ACT, per chunk.
# ch0 front-loads DVE while va hasn't started (PE outruns a single ACT);
# last chunk keeps the final 8 tiles on ACT (direct-Z tail).
DVE_JTS = {
    0: {1, 3, 9, 16, 24, 28},
    1: {2, 7, 12, 17, 22, 27, 30},
    2: {2, 7, 12, 17, 22, 27, 30},
    3: {2, 7, 12, 17, 20, 23},
}

_CACHE = {}


def _build():
    nc = bacc.Bacc("TRN2", target_bir_lowering=False, debug=False, num_devices=B)

    pose_d = nc.dram_tensor("pose", [C, L], F32, kind="ExternalInput").ap()
    posebf_d = nc.dram_tensor("posebf", [C, L], BF16, kind="ExternalInput").ap()
    idbf_d = nc.dram_tensor("idbf", [C, L], BF16, kind="ExternalInput").ap()
    wt_d = nc.dram_tensor("wt", [C, 3 * C], BF16, kind="ExternalInput").ap()
    bq_d = nc.dram_tensor("bq", [C, 1], F32, kind="ExternalInput").ap()
    bk_d = nc.dram_tensor("bk", [C, 1], F32, kind="ExternalInput").ap()
    bfin_d = nc.dram_tensor("bfin", [C, 1], F32, kind="ExternalInput").ap()
    gam_d = nc.dram_tensor("gam", [C, 1], F32, kind="ExternalInput").ap()
    out_d = nc.dram_tensor("out", [C, L], F32, kind="ExternalOutput").ap()

    with tile.TileContext(nc) as tc:
        with tc.tile_pool(name="res", bufs=1) as res:
            # ---- input DMAs spread over queues ----
            # sync:   wt, idbf0, idbf1, bq, bk   | vt transposes, out halves
            # scalar: posebf0, idbf2, idbf3, bfin, gam | vt transposes, outs
            # gpsimd: posebf1..3, pose0..3
            wt_sb = res.tile([C, 3 * C], BF16)
            nc.sync.dma_start(wt_sb, wt_d)

            def chunk_tiles(prefix, dtype):
                return [res.tile([C, CHUNK], dtype, name=f"{prefix}{i}")
                        for i in range(NCH)]

            pose_t = chunk_tiles("pose", F32)
            posebf_t = chunk_tiles("posebf", BF16)
            idbf_t = chunk_tiles("idbf", BF16)
            q_t = chunk_tiles("q", BF16)
            k_t = chunk_tiles("k", BF16)
            v_t = chunk_tiles("v", BF16)
            vt_t = chunk_tiles("vt", BF16)   # [j (partition), jt*128 + c]

            def dslice(ch):
                return slice(ch * CHUNK, (ch + 1) * CHUNK)

            nc.sync.dma_start(idbf_t[0], idbf_d[:, dslice(0)])
            nc.sync.dma_start(idbf_t[2], idbf_d[:, dslice(2)])
            bq_sb = res.tile([C, 1], F32)
            bk_sb = res.tile([C, 1], F32)
            nc.sync.dma_start(bq_sb, bq_d)
            nc.sync.dma_start(bk_sb, bk_d)

            nc.scalar.dma_start(posebf_t[0], posebf_d[:, dslice(0)])
            nc.scalar.dma_start(idbf_t[1], idbf_d[:, dslice(1)])
            nc.scalar.dma_start(idbf_t[3], idbf_d[:, dslice(3)])
            bfin_sb = res.tile([C, 1], F32)
            gam_sb = res.tile([C, 1], F32)
            nc.scalar.dma_start(bfin_sb, bfin_d)
            nc.scalar.dma_start(gam_sb, gam_d)

            for ch in range(1, NCH):
                nc.gpsimd.dma_start(posebf_t[ch], posebf_d[:, dslice(ch)])
            for ch in range(NCH):
                nc.gpsimd.dma_start(pose_t[ch], pose_d[:, dslice(ch)])

            # bf16 1.0 const: memset at Bass init (before user code) — the
            # Z-matmul ones vector and a dependency-free warmup operand.
            ones_sb = nc.const_aps.tensor(1.0, (C, 1), BF16)
            onesr_sb = res.tile([1, C], F32)
            nc.vector.memset(onesr_sb, 1.0)

            wqT = wt_sb[:, 0:C]
            wkT = wt_sb[:, C:2 * C]
            wvT = wt_sb[:, 2 * C:3 * C]

            def conv_mms(psum, w, src):
                for h in range(CHUNK // 512):
                    hs = slice(h * 512, (h + 1) * 512)
                    nc.tensor.matmul(psum[:, hs], lhsT=w, rhs=src[:, hs],
                                     start=True, stop=True)

            # ---- warmup + head convs share one PSUM pool (separate tags)
            # so the convs don't wait for a warmup-pool close. Warm MMs:
            # stage 1 on the const AP (zero deps — starts the instant the
            # PE queue opens, flips HAM to 8/8), stage 2 wider, covering
            # until the input DMAs land. Convs v0,q0,k0..k3 follow (energy
            # ch0 needs every k chunk).
            warm_sb = res.tile([C, 384], BF16)
            nc.vector.memset(warm_sb, 0.0)
            with tc.tile_pool(name="conv_ps", bufs=3, space="PSUM") as conv_ps:
                wp1 = conv_ps.tile([1, 1], F32, tag="w1", bufs=1)
                for _ in range(60):
                    nc.tensor.matmul(wp1, lhsT=ones_sb, rhs=ones_sb,
                                     start=True, stop=True)
                wp2 = conv_ps.tile([1, 384], F32, tag="w2", bufs=1)
                for _ in range(WARM_MMS):
                    nc.tensor.matmul(wp2, lhsT=ones_sb, rhs=warm_sb,
                                     start=True, stop=True)
                vp0 = conv_ps.tile([C, CHUNK], F32, tag="cv", name="vp0")
                conv_mms(vp0, wvT, posebf_t[0])
                # v0 drained in halves so both HWDGE xbars transpose in parallel
                nc.scalar.copy(v_t[0][:, 0:512], vp0[:, 0:512])
                nc.scalar.copy(v_t[0][:, 512:1024], vp0[:, 512:1024])
                nc.sync.dma_start_transpose(
                    vt_t[0][:, 0:512].rearrange("p (t c) -> p t c", c=C),
                    v_t[0][:, 0:512])
                nc.scalar.dma_start_transpose(
                    vt_t[0][:, 512:1024].rearrange("p (t c) -> p t c", c=C),
                    v_t[0][:, 512:1024])
                qp0 = conv_ps.tile([C, CHUNK], F32, tag="cv", name="qp0")
                conv_mms(qp0, wqT, posebf_t[0])
                nc.vector.tensor_scalar_add(q_t[0], qp0, bq_sb)
                kp0 = conv_ps.tile([C, CHUNK], F32, tag="cv", name="kp0")
                conv_mms(kp0, wkT, idbf_t[0])
                nc.scalar.activation(k_t[0], kp0, AF.Identity, bias=bk_sb)

            # ---- attention ----
            with (
                tc.tile_pool(name="et_ps", bufs=2, space="PSUM") as et_ps,
                tc.tile_pool(name="va_ps", bufs=1, space="PSUM") as va_ps,
                tc.tile_pool(name="z_ps", bufs=1, space="PSUM") as z_ps,
                tc.tile_pool(name="pt_sb", bufs=6) as pt_pool,
                tc.tile_pool(name="qs_sb", bufs=2) as qs_pool,
                tc.tile_pool(name="nrm", bufs=2) as nrm,
                tc.tile_pool(name="outb", bufs=2) as outb,
                tc.tile_pool(name="dramp", bufs=2, space="DRAM") as dramp,
            ):
                for ch in range(NCH):
                    i0 = ch * CHUNK
                    isl = slice(i0, i0 + CHUNK)
                    last = ch == NCH - 1
                    dve_set = DVE_JTS[ch]
                    va = va_ps.tile([C, CHUNK], F32)
                    z = z_ps.tile([1, CHUNK], F32)
                    pairs = {}      # pair idx -> [C, 2*CHUNK] tile
                    ssums = {}      # group idx -> [C, CHUNK] presum (8 jt)
                    asums = {}      # group idx -> [C, 2*CHUNK] A-partial

                    def skew_at(j):
                        if not last:
                            return SKEW
                        return max(2, min(SKEW, NJT + 2 - j))

                    lag_ptr = 0
                    n_extra = 2 if last else SKEW
                    for jt in range(NJT + n_extra):
                        if jt < NJT:
                            ksl = slice((jt % JPC) * 128, (jt % JPC + 1) * 128)
                            et = et_ps.tile([C, CHUNK], F32, tag="et")
                            for h in range(CHUNK // 512):
                                hs = slice(h * 512, (h + 1) * 512)
                                nc.tensor.matmul(
                                    et[:, hs], lhsT=k_t[jt // JPC][:, ksl],
                                    rhs=q_t[ch][:, hs],
                                    start=True, stop=True)
                            pr = jt // 2
                            if jt % 2 == 0:
                                pairs[pr] = pt_pool.tile([C, 2 * CHUNK], BF16,
                                                         name="pt")
                            half = pairs[pr][:, (jt % 2) * CHUNK:
                                             (jt % 2 + 1) * CHUNK]
                            if jt in dve_set:
                                nc.vector.tensor_scalar(
                                    out=half.bitcast(I16), in0=et,
                                    scalar1=SCHRAUD_A, scalar2=SCHRAUD_B,
                                    op0=ALU.mult, op1=ALU.add)
                            else:
                                nc.scalar.activation(half, et, AF.Exp)

                            # ch1-3 k/v/q convs ride chunk 0's stream (energy
                            # needs k_X by jt=8X; v/q by the next chunk)
                            if ch == 0 and jt in (4, 12, 20):
                                cc = jt // 8 + 1
                                kp = et_ps.tile([C, CHUNK], F32, tag="et",
                                                name=f"kp{cc}")
                                conv_mms(kp, wkT, idbf_t[cc])
                                nc.scalar.activation(k_t[cc], kp, AF.Identity,
                                                     bias=bk_sb)
                            if ch == 0 and jt in (5, 13, 21):
                                cc = jt // 8 + 1
                                vp = et_ps.tile([C, CHUNK], F32, tag="et",
                                                name=f"vp{cc}")
                                conv_mms(vp, wvT, posebf_t[cc])
                                nc.scalar.copy(v_t[cc], vp)
                                eng = nc.sync if cc != 2 else nc.scalar
                                eng.dma_start_transpose(
                                    vt_t[cc].rearrange("p (t c) -> p t c", c=C),
                                    v_t[cc])
                            if ch == 0 and jt in (6, 14, 22):
                                cc = jt // 8 + 1
                                qp = et_ps.tile([C, CHUNK], F32, tag="et",
                                                name=f"qp{cc}")
                                conv_mms(qp, wqT, posebf_t[cc])
                                nc.vector.tensor_scalar_add(q_t[cc], qp, bq_sb)

                            # Z presums on pair tiles (skip tail tiles of
                            # the last chunk: those go direct)
                            g = jt // 8
                            if jt % 8 == 3 and not (last and g == 3):
                                a = qs_pool.tile([C, 2 * CHUNK], BF16, tag="a")
                                nc.vector.tensor_add(a, pairs[4 * g],
                                                     pairs[4 * g + 1])
                                asums[g] = a
                            if jt % 8 == 7 and not (last and g == 3):
                                b_ = qs_pool.tile([C, 2 * CHUNK], BF16, tag="b")
                                nc.vector.tensor_add(b_, pairs[4 * g + 2],
                                                     pairs[4 * g + 3])
                                cs = qs_pool.tile([C, 2 * CHUNK], BF16, tag="c")
                                nc.vector.tensor_add(cs, asums.pop(g), b_)
                                s = qs_pool.tile([C, CHUNK], BF16, tag="s",
                                                 bufs=3)
                                nc.vector.tensor_add(s, cs[:, 0:CHUNK],
                                                     cs[:, CHUNK:2 * CHUNK])
                                ssums[g] = s
                            if last and jt == 27:
                                # partial group 3: jts 24-27 presummed
                                a = qs_pool.tile([C, 2 * CHUNK], BF16, tag="a")
                                nc.vector.tensor_add(a, pairs[12], pairs[13])
                                s = qs_pool.tile([C, CHUNK], BF16, tag="s",
                                                 bufs=3)
                                nc.vector.tensor_add(s, a[:, 0:CHUNK],
                                                     a[:, CHUNK:2 * CHUNK])
                                ssums[3] = s
                            # pose' = pose + gamma*bv, on ACT (free affine)
                            if jt == 29:
                                nc.scalar.activation(pose_t[ch], pose_t[ch],
                                                     AF.Identity, bias=bfin_sb)

                        while lag_ptr <= min(jt - skew_at(jt), NJT - 1):
                            lag = lag_ptr
                            lag_ptr += 1
                            vsl = slice((lag % JPC) * 128,
                                        (lag % JPC + 1) * 128)
                            phalf = pairs[lag // 2][:, (lag % 2) * CHUNK:
                                                    (lag % 2 + 1) * CHUNK]
                            for h in range(CHUNK // 512):
                                hs = slice(h * 512, (h + 1) * 512)
                                nc.tensor.matmul(
                                    va[:, hs], lhsT=vt_t[lag // JPC][:, vsl],
                                    rhs=phalf[:, hs],
                                    start=(lag == 0),
                                    stop=(lag == NJT - 1))
                            lg = lag // 8
                            if last and lag >= 28:
                                # tail: direct M=1 Z matmuls, no DVE chain
                                for h in range(CHUNK // 512):
                                    hs = slice(h * 512, (h + 1) * 512)
                                    nc.tensor.matmul(z[0:1, hs], lhsT=ones_sb,
                                                     rhs=phalf[:, hs],
                                                     start=False,
                                                     stop=(lag == NJT - 1))
                            elif lag % 8 == 7 or (last and lag == 27):
                                s = ssums.pop(lg)
                                for h in range(CHUNK // 512):
                                    hs = slice(h * 512, (h + 1) * 512)
                                    nc.tensor.matmul(
                                        z[0:1, hs], lhsT=ones_sb,
                                        rhs=s[:, hs], start=(lg == 0),
                                        stop=(not last and lg == 3))

                    # free the va PSUM bank quickly, then normalize from SBUF
                    va_sb = nrm.tile([C, CHUNK], F32, tag="va_sb")
                    nc.vector.tensor_copy(va_sb, va)
                    rz = nrm.tile([1, CHUNK], F32, tag="rz")
                    if not last:
                        nc.vector.reciprocal_approx_fast(rz, z)
                        # broadcast 1/Z across partitions via DRAM round-trip
                        # on the idle gpsimd queue (hidden under next chunk)
                        zd = dramp.tile([1, CHUNK], F32)
                        nc.gpsimd.dma_start(zd, rz)
                        rzb = nrm.tile([C, CHUNK], F32, tag="rzb")
                        nc.gpsimd.dma_start(rzb, zd.to_broadcast([C, CHUNK]))
                        t = nrm.tile([C, CHUNK], F32, tag="t")
                        nc.vector.tensor_mul(t, va_sb, rzb)
                        o = outb.tile([C, CHUNK], F32)
                        nc.vector.scalar_tensor_tensor(
                            o, in0=t, scalar=gam_sb, in1=pose_t[ch],
                            op0=ALU.mult, op1=ALU.add)
                        nc.sync.dma_start(out_d[:, i0:i0 + 512], o[:, 0:512])
                        nc.scalar.dma_start(out_d[:, i0 + 512:i0 + CHUNK],
                                            o[:, 512:1024])
                    else:
                        # tail: PE broadcasts 1/Z (idle now); normalize in
                        # two pipelined 512-halves so the first out-DMA
                        # overlaps the second half's compute
                        rzb = et_ps.tile([C, CHUNK], F32, tag="et",
                                         name="rzb_ps")
                        t = nrm.tile([C, CHUNK], F32, tag="t")
                        o = outb.tile([C, CHUNK], F32)
                        for h in range(CHUNK // 512):
                            hs = slice(h * 512, (h + 1) * 512)
                            ihs = slice(i0 + h * 512, i0 + (h + 1) * 512)
                            nc.vector.reciprocal_approx_fast(rz[0:1, hs],
                                                             z[0:1, hs])
                            nc.tensor.matmul(rzb[:, hs], lhsT=onesr_sb,
                                             rhs=rz[0:1, hs],
                                             start=True, stop=True)
                            nc.vector.tensor_mul(t[:, hs], va_sb[:, hs],
                                                 rzb[:, hs])
                            nc.vector.scalar_tensor_tensor(
                                o[:, hs], in0=t[:, hs], scalar=gam_sb,
                                in1=pose_t[ch][:, hs],
                                op0=ALU.mult, op1=ALU.add)
                            eng = nc.sync if h == 0 else nc.scalar
                            eng.dma_start(out_d[:, ihs], o[:, hs])

    nc.compile()
    return nc


def _get_nc():
    if "nc" not in _CACHE:
        _CACHE["nc"] = _build()
    return _CACHE["nc"]


def kernel(pose_f, id_f, Wq, bq, Wk, bk, Wv, bv, gamma, **run_kwargs):
    pose_f = np.asarray(pose_f, dtype=np.float32)
    id_f = np.asarray(id_f, dtype=np.float32)
    Wq = np.asarray(Wq, dtype=np.float32)
    Wk = np.asarray(Wk, dtype=np.float32)
    Wv = np.asarray(Wv, dtype=np.float32)
    bq = np.asarray(bq, dtype=np.float32)
    bk = np.asarray(bk, dtype=np.float32)
    bv = np.asarray(bv, dtype=np.float32)
    g = float(np.asarray(gamma, dtype=np.float32).reshape(-1)[0])

    bf = ml_dtypes.bfloat16
    wt = np.concatenate([Wq.T, Wk.T, Wv.T], axis=1).astype(bf)  # [C_in, 3C]
    posebf = pose_f.astype(bf)
    idbf = id_f.astype(bf)
    bq_c = np.ascontiguousarray(bq.reshape(C, 1))
    bk_c = np.ascontiguousarray(bk.reshape(C, 1))
    bfin = np.ascontiguousarray((g * bv).reshape(C, 1).astype(np.float32))
    gam = np.full((C, 1), g, dtype=np.float32)

    in_maps = []
    for b in range(B):
        in_maps.append({
            "pose": pose_f[b],
            "posebf": posebf[b],
            "idbf": idbf[b],
            "wt": wt,
            "bq": bq_c,
            "bk": bk_c,
            "bfin": bfin,
            "gam": gam,
        })

    nc = _get_nc()
    res = run_bass_kernel_spmd(nc, in_maps, core_ids=list(range(B)), **run_kwargs)
    out = np.stack([res.results[b]["out"] for b in range(B)], axis=0)
    if run_kwargs:
        _CACHE["last_result"] = res
    return out


# revision 18
# speedup vs baseline: 1.0428x; 1.0174x over previous
"""Trainium2 Bass kernel for nn_CGPBlock (attention block with 1x1-conv QKV).

Reference computation (per batch b):
    q = Wq @ pose + bq; k = Wk @ id + bk; v = Wv @ pose + bv     # [C, L]
    energy[i, j] = sum_c q[c, i] k[c, j]                          # [L, L]
    attn = softmax_j(energy)
    va[c, i] = sum_j v[c, j] attn[i, j]
    out = pose + gamma * va

Sharding: data-parallel over batch, B=8 batches -> 8 NeuronCores (SPMD, no
collectives). Per core: C=128 fills the partition dim, L=4096.

Device algorithm (per core, matmuls bf16 with fp32 PSUM accumulate):
  - For each i-chunk (1024), loop j-tiles (128): eT[j,i] = k_jt.T @ q_chunk
    (PSUM), then softmax numerator p = exp(eT) WITHOUT max subtraction
    (|E| < 40 in practice, fp32/bf16 safe), then va += vt_jt.T @ p (PSUM
    accumulate) and Z (softmax denominators) via ones.T @ (presummed p).
  - exp is split across TWO engines to halve the softmax-stage cost:
    ACT does real exp; DVE computes a Schraudolph approximation in one
    tensor_scalar op: int16(E*(128/ln2) + 16248.75) IS the bf16 bit
    pattern of exp(E) with a linear-mantissa approx (~1.8% rms error,
    cancels in va/Z normalization; tolerance is 2e-2).
  - p j-tile PAIRS share one [C, 2048] tile so Z presums run as wide DVE
    tensor_tensor adds (3x 2048-wide + 1 fold per 8 j-tiles), then one
    M=1 matmul per 8 j-tiles accumulates Z in PSUM.
  - Startup: PE clock-gate warmed by garbage matmuls on an uninitialized
    SBUF tile (no DMA/DVE dependency -> starts ~0.3us, HAM warm by ~4us).
    Inputs spread across 4 DMA queues (sync/scalar HWDGE, gpsimd SWDGE)
    so the first convs start ~10us and energy ~15us (vs 25us serial).
    vt (v transposed) for chunk 0 is produced as two half-transposes on
    the two HWDGE xbars in parallel to beat the first va matmul.
  - ch1-3 v/q convs + drains are interleaved INTO chunk 0's attention
    stream (PSUM tiles recycled from the energy pool) so all k convs can
    run up front (energy needs every k chunk) without extra PSUM.
  - Tail: last chunk's Z ends in 4 direct M=1 matmuls (skew decayed to 2)
    and the normalize runs once at full width with a PE-broadcast 1/Z,
    output split across 4 DMA queues.
"""

import numpy as np
import ml_dtypes

import concourse.bacc as bacc
import concourse.tile as tile
from concourse import mybir
from concourse.bass_utils import run_bass_kernel_spmd

F32 = mybir.dt.float32
BF16 = mybir.dt.bfloat16
I16 = mybir.dt.int16
AF = mybir.ActivationFunctionType
ALU = mybir.AluOpType

B, C, L = 8, 128, 4096
CHUNK = 1024                # i-chunk width
NCH = L // CHUNK            # 4 chunks
NJT = L // 128              # 32 j-tiles per chunk
JPC = CHUNK // 128          # j-tiles per k/v chunk tile
SKEW = 8                    # software pipeline depth (PE runs ahead of exp)
WARM_MMS = 12               # garbage matmuls covering the input-DMA window

SCHRAUD_A = 128.0 / float(np.log(2.0))   # 184.664965...
SCHRAUD_B = 16248.75                     # round-to-nearest calibrated

# j-tiles whose exp runs on DVE (Schraudolph) instead of ACT, per chunk.
# ch0 front-loads DVE while va hasn't started (PE outruns a single ACT);
# last chunk keeps the final 8 tiles on ACT (direct-Z tail).
DVE_JTS = {
    0: {1, 3, 9, 16, 24, 28},
    1: {2, 7, 12, 17, 22, 27, 30},
    2: {2, 7, 12, 17, 22, 27, 30},
    3: {2, 7, 12, 17, 20, 23},
}

_CACHE = {}


def _build():
    nc = bacc.Bacc("TRN2", target_bir_lowering=False, debug=False, num_devices=B)

    pose_d = nc.dram_tensor("pose", [C, L], F32, kind="ExternalInput").ap()
    posebf_d = nc.dram_tensor("posebf", [C, L], BF16, kind="ExternalInput").ap()
    idbf_d = nc.dram_tensor("idbf", [C, L], BF16, kind="ExternalInput").ap()
    wt_d = nc.dram_tensor("wt", [C, 3 * C], BF16, kind="ExternalInput").ap()
    bq_d = nc.dram_tensor("bq", [C, 1], F32, kind="ExternalInput").ap()
    bk_d = nc.dram_tensor("bk", [C, 1], F32, kind="ExternalInput").ap()
    bfin_d = nc.dram_tensor("bfin", [C, 1], F32, kind="ExternalInput").ap()
    gam_d = nc.dram_tensor("gam", [C, 1], F32, kind="ExternalInput").ap()
    out_d = nc.dram_tensor("out", [C, L], F32, kind="ExternalOutput").ap()

    with tile.TileContext(nc) as tc:
        with tc.tile_pool(name="res", bufs=1) as res:
            # ---- input DMAs spread over queues ----
            # sync:   wt, idbf0, idbf1, bq, bk   | vt transposes, out halves
            # scalar: posebf0, idbf2, idbf3, bfin, gam | vt transposes, outs
            # gpsimd: posebf1..3, pose0..3
            wt_sb = res.tile([C, 3 * C], BF16)
            nc.sync.dma_start(wt_sb, wt_d)

            def chunk_tiles(prefix, dtype):
                return [res.tile([C, CHUNK], dtype, name=f"{prefix}{i}")
                        for i in range(NCH)]

            pose_t = chunk_tiles("pose", F32)
            posebf_t = chunk_tiles("posebf", BF16)
            idbf_t = chunk_tiles("idbf", BF16)
            q_t = chunk_tiles("q", BF16)
            k_t = chunk_tiles("k", BF16)
            v_t = chunk_tiles("v", BF16)
            vt_t = chunk_tiles("vt", BF16)   # [j (partition), jt*128 + c]

            def dslice(ch):
                return slice(ch * CHUNK, (ch + 1) * CHUNK)

            # Window 1: only the head-critical tiles — the DMA engines
            # fair-share bandwidth across ALL in-flight transfers, so
            # late-needed data must not be enqueued yet.
            nc.sync.dma_start(idbf_t[0], idbf_d[:, dslice(0)])
            bq_sb = res.tile([C, 1], F32)
            bk_sb = res.tile([C, 1], F32)
            nc.sync.dma_start(bq_sb, bq_d)
            nc.sync.dma_start(bk_sb, bk_d)

            nc.scalar.dma_start(posebf_t[0], posebf_d[:, dslice(0)])
            nc.scalar.dma_start(idbf_t[1], idbf_d[:, dslice(1)])
            bfin_sb = res.tile([C, 1], F32)
            gam_sb = res.tile([C, 1], F32)
            nc.scalar.dma_start(bfin_sb, bfin_d)
            nc.scalar.dma_start(gam_sb, gam_d)

            nc.gpsimd.dma_start(posebf_t[1], posebf_d[:, dslice(1)])

            # bf16 1.0 const: memset at Bass init (before user code) — the
            # Z-matmul ones vector and a dependency-free warmup operand.
            ones_sb = nc.const_aps.tensor(1.0, (C, 1), BF16)
            onesr_sb = res.tile([1, C], F32)
            nc.vector.memset(onesr_sb, 1.0)

            wqT = wt_sb[:, 0:C]
            wkT = wt_sb[:, C:2 * C]
            wvT = wt_sb[:, 2 * C:3 * C]

            def conv_mms(psum, w, src):
                for h in range(CHUNK // 512):
                    hs = slice(h * 512, (h + 1) * 512)
                    nc.tensor.matmul(psum[:, hs], lhsT=w, rhs=src[:, hs],
                                     start=True, stop=True)

            # ---- warmup + head convs share one PSUM pool (separate tags)
            # so the convs don't wait for a warmup-pool close. Warm MMs:
            # stage 1 on the const AP (zero deps — starts the instant the
            # PE queue opens, flips HAM to 8/8), stage 2 wider, covering
            # until the input DMAs land. Convs v0,q0,k0..k3 follow (energy
            # ch0 needs every k chunk).
            warm_sb = res.tile([C, 384], BF16)
            nc.vector.memset(warm_sb, 0.0)
            with tc.tile_pool(name="conv_ps", bufs=3, space="PSUM") as conv_ps:
                wp1 = conv_ps.tile([1, 1], F32, tag="w1", bufs=1)
                for _ in range(60):
                    nc.tensor.matmul(wp1, lhsT=ones_sb, rhs=ones_sb,
                                     start=True, stop=True)
                wp2 = conv_ps.tile([1, 384], F32, tag="w2", bufs=1)
                for _ in range(WARM_MMS):
                    nc.tensor.matmul(wp2, lhsT=ones_sb, rhs=warm_sb,
                                     start=True, stop=True)
                vp0 = conv_ps.tile([C, CHUNK], F32, tag="cv", name="vp0")
                conv_mms(vp0, wvT, posebf_t[0])
                # v0 drained in halves so both HWDGE xbars transpose in parallel
                nc.scalar.copy(v_t[0][:, 0:512], vp0[:, 0:512])
                nc.scalar.copy(v_t[0][:, 512:1024], vp0[:, 512:1024])
                nc.sync.dma_start_transpose(
                    vt_t[0][:, 0:512].rearrange("p (t c) -> p t c", c=C),
                    v_t[0][:, 0:512])
                nc.scalar.dma_start_transpose(
                    vt_t[0][:, 512:1024].rearrange("p (t c) -> p t c", c=C),
                    v_t[0][:, 512:1024])
                # Window 2: queued behind the (v0-drain-gated) transpose
                # triggers, so these start ~4us later than window 1
                nc.sync.dma_start(idbf_t[2], idbf_d[:, dslice(2)])
                nc.sync.dma_start(pose_t[0], pose_d[:, dslice(0)])
                nc.scalar.dma_start(posebf_t[2], posebf_d[:, dslice(2)])
                nc.scalar.dma_start(idbf_t[3], idbf_d[:, dslice(3)])
                nc.scalar.dma_start(posebf_t[3], posebf_d[:, dslice(3)])
                nc.scalar.dma_start(pose_t[2], pose_d[:, dslice(2)])
                qp0 = conv_ps.tile([C, CHUNK], F32, tag="cv", name="qp0")
                conv_mms(qp0, wqT, posebf_t[0])
                nc.vector.tensor_scalar_add(q_t[0], qp0, bq_sb)
                kp0 = conv_ps.tile([C, CHUNK], F32, tag="cv", name="kp0")
                conv_mms(kp0, wkT, idbf_t[0])
                nc.scalar.activation(k_t[0], kp0, AF.Identity, bias=bk_sb)

            # Window 3 (pose fp32, needed only at the first normalize
            # ~40us in): gpsimd stalls on a guard copy of q_t[1] (written
            # by the jt6 conv insert) before firing these transfers.
            guard = res.tile([1, 8], BF16)
            nc.gpsimd.tensor_copy(guard, q_t[1][0:1, 0:8])
            nc.gpsimd.dma_start(pose_t[1], pose_d[:, dslice(1)])
            nc.gpsimd.dma_start(pose_t[3], pose_d[:, dslice(3)])

            # ---- attention ----
            with (
                tc.tile_pool(name="et_ps", bufs=2, space="PSUM") as et_ps,
                tc.tile_pool(name="va_ps", bufs=1, space="PSUM") as va_ps,
                tc.tile_pool(name="z_ps", bufs=1, space="PSUM") as z_ps,
                tc.tile_pool(name="pt_sb", bufs=6) as pt_pool,
                tc.tile_pool(name="qs_sb", bufs=2) as qs_pool,
                tc.tile_pool(name="nrm", bufs=2) as nrm,
                tc.tile_pool(name="outb", bufs=2) as outb,
                tc.tile_pool(name="dramp", bufs=2, space="DRAM") as dramp,
            ):
                for ch in range(NCH):
                    i0 = ch * CHUNK
                    isl = slice(i0, i0 + CHUNK)
                    last = ch == NCH - 1
                    dve_set = DVE_JTS[ch]
                    va = va_ps.tile([C, CHUNK], F32)
                    z = z_ps.tile([1, CHUNK], F32)
                    pairs = {}      # pair idx -> [C, 2*CHUNK] tile
                    ssums = {}      # group idx -> [C, CHUNK] presum (8 jt)
                    asums = {}      # group idx -> [C, 2*CHUNK] A-partial

                    def skew_at(j):
                        if not last:
                            return SKEW
                        return max(2, min(SKEW, NJT + 2 - j))

                    lag_ptr = 0
                    n_extra = 2 if last else SKEW
                    for jt in range(NJT + n_extra):
                        if jt < NJT:
                            ksl = slice((jt % JPC) * 128, (jt % JPC + 1) * 128)
                            et = et_ps.tile([C, CHUNK], F32, tag="et")
                            for h in range(CHUNK // 512):
                                hs = slice(h * 512, (h + 1) * 512)
                                nc.tensor.matmul(
                                    et[:, hs], lhsT=k_t[jt // JPC][:, ksl],
                                    rhs=q_t[ch][:, hs],
                                    start=True, stop=True)
                            pr = jt // 2
                            if jt % 2 == 0:
                                pairs[pr] = pt_pool.tile([C, 2 * CHUNK], BF16,
                                                         name="pt")
                            half = pairs[pr][:, (jt % 2) * CHUNK:
                                             (jt % 2 + 1) * CHUNK]
                            if jt in dve_set:
                                nc.vector.tensor_scalar(
                                    out=half.bitcast(I16), in0=et,
                                    scalar1=SCHRAUD_A, scalar2=SCHRAUD_B,
                                    op0=ALU.mult, op1=ALU.add)
                            else:
                                nc.scalar.activation(half, et, AF.Exp)

                            # ch1-3 k/v/q convs ride chunk 0's stream (energy
                            # needs k_X by jt=8X; v/q by the next chunk)
                            if ch == 0 and jt in (4, 12, 20):
                                cc = jt // 8 + 1
                                kp = et_ps.tile([C, CHUNK], F32, tag="et",
                                                name=f"kp{cc}")
                                conv_mms(kp, wkT, idbf_t[cc])
                                nc.scalar.activation(k_t[cc], kp, AF.Identity,
                                                     bias=bk_sb)
                            if ch == 0 and jt in (5, 13, 21):
                                cc = jt // 8 + 1
                                vp = et_ps.tile([C, CHUNK], F32, tag="et",
                                                name=f"vp{cc}")
                                conv_mms(vp, wvT, posebf_t[cc])
                                nc.scalar.copy(v_t[cc], vp)
                                eng = nc.sync if cc != 2 else nc.scalar
                                eng.dma_start_transpose(
                                    vt_t[cc].rearrange("p (t c) -> p t c", c=C),
                                    v_t[cc])
                            if ch == 0 and jt in (6, 14, 22):
                                cc = jt // 8 + 1
                                qp = et_ps.tile([C, CHUNK], F32, tag="et",
                                                name=f"qp{cc}")
                                conv_mms(qp, wqT, posebf_t[cc])
                                nc.vector.tensor_scalar_add(q_t[cc], qp, bq_sb)

                            # Z presums on pair tiles (skip tail tiles of
                            # the last chunk: those go direct)
                            g = jt // 8
                            if jt % 8 == 3 and not (last and g == 3):
                                a = qs_pool.tile([C, 2 * CHUNK], BF16, tag="a")
                                nc.vector.tensor_add(a, pairs[4 * g],
                                                     pairs[4 * g + 1])
                                asums[g] = a
                            if jt % 8 == 7 and not (last and g == 3):
                                b_ = qs_pool.tile([C, 2 * CHUNK], BF16, tag="b")
                                nc.vector.tensor_add(b_, pairs[4 * g + 2],
                                                     pairs[4 * g + 3])
                                cs = qs_pool.tile([C, 2 * CHUNK], BF16, tag="c")
                                nc.vector.tensor_add(cs, asums.pop(g), b_)
                                s = qs_pool.tile([C, CHUNK], BF16, tag="s",
                                                 bufs=3)
                                nc.vector.tensor_add(s, cs[:, 0:CHUNK],
                                                     cs[:, CHUNK:2 * CHUNK])
                                ssums[g] = s
                            if last and jt == 27:
                                # partial group 3: jts 24-27 presummed
                                a = qs_pool.tile([C, 2 * CHUNK], BF16, tag="a")
                                nc.vector.tensor_add(a, pairs[12], pairs[13])
                                s = qs_pool.tile([C, CHUNK], BF16, tag="s",
                                                 bufs=3)
                                nc.vector.tensor_add(s, a[:, 0:CHUNK],
                                                     a[:, CHUNK:2 * CHUNK])
                                ssums[3] = s
                            # pose' = pose + gamma*bv, on ACT (free affine)
                            if jt == 29:
                                nc.scalar.activation(pose_t[ch], pose_t[ch],
                                                     AF.Identity, bias=bfin_sb)

                        while lag_ptr <= min(jt - skew_at(jt), NJT - 1):
                            lag = lag_ptr
                            lag_ptr += 1
                            vsl = slice((lag % JPC) * 128,
                                        (lag % JPC + 1) * 128)
                            phalf = pairs[lag // 2][:, (lag % 2) * CHUNK:
                                                    (lag % 2 + 1) * CHUNK]
                            for h in range(CHUNK // 512):
                                hs = slice(h * 512, (h + 1) * 512)
                                nc.tensor.matmul(
                                    va[:, hs], lhsT=vt_t[lag // JPC][:, vsl],
                                    rhs=phalf[:, hs],
                                    start=(lag == 0),
                                    stop=(lag == NJT - 1))
                            lg = lag // 8
                            if last and lag >= 28:
                                # tail: direct M=1 Z matmuls, no DVE chain
                                for h in range(CHUNK // 512):
                                    hs = slice(h * 512, (h + 1) * 512)
                                    nc.tensor.matmul(z[0:1, hs], lhsT=ones_sb,
                                                     rhs=phalf[:, hs],
                                                     start=False,
                                                     stop=(lag == NJT - 1))
                            elif lag % 8 == 7 or (last and lag == 27):
                                s = ssums.pop(lg)
                                for h in range(CHUNK // 512):
                                    hs = slice(h * 512, (h + 1) * 512)
                                    nc.tensor.matmul(
                                        z[0:1, hs], lhsT=ones_sb,
                                        rhs=s[:, hs], start=(lg == 0),
                                        stop=(not last and lg == 3))

                    # free the va PSUM bank quickly, then normalize from SBUF
                    va_sb = nrm.tile([C, CHUNK], F32, tag="va_sb")
                    nc.vector.tensor_copy(va_sb, va)
                    rz = nrm.tile([1, CHUNK], F32, tag="rz")
                    if not last:
                        nc.vector.reciprocal_approx_fast(rz, z)
                        # broadcast 1/Z across partitions via DRAM round-trip
                        # on the idle gpsimd queue (hidden under next chunk)
                        zd = dramp.tile([1, CHUNK], F32)
                        nc.gpsimd.dma_start(zd, rz)
                        rzb = nrm.tile([C, CHUNK], F32, tag="rzb")
                        nc.gpsimd.dma_start(rzb, zd.to_broadcast([C, CHUNK]))
                        t = nrm.tile([C, CHUNK], F32, tag="t")
                        nc.vector.tensor_mul(t, va_sb, rzb)
                        o = outb.tile([C, CHUNK], F32)
                        nc.vector.scalar_tensor_tensor(
                            o, in0=t, scalar=gam_sb, in1=pose_t[ch],
                            op0=ALU.mult, op1=ALU.add)
                        nc.sync.dma_start(out_d[:, i0:i0 + 512], o[:, 0:512])
                        nc.scalar.dma_start(out_d[:, i0 + 512:i0 + CHUNK],
                                            o[:, 512:1024])
                    else:
                        # tail: PE broadcasts 1/Z (idle now); normalize in
                        # two pipelined 512-halves so the first out-DMA
                        # overlaps the second half's compute
                        rzb = et_ps.tile([C, CHUNK], F32, tag="et",
                                         name="rzb_ps")
                        t = nrm.tile([C, CHUNK], F32, tag="t")
                        o = outb.tile([C, CHUNK], F32)
                        for h in range(CHUNK // 512):
                            hs = slice(h * 512, (h + 1) * 512)
                            ihs = slice(i0 + h * 512, i0 + (h + 1) * 512)
                            nc.vector.reciprocal_approx_fast(rz[0:1, hs],
                                                             z[0:1, hs])
                            nc.tensor.matmul(rzb[:, hs], lhsT=onesr_sb,
                                             rhs=rz[0:1, hs],
                                             start=True, stop=True)
                            nc.vector.tensor_mul(t[:, hs], va_sb[:, hs],
                                                 rzb[:, hs])
                            nc.vector.scalar_tensor_tensor(
                                o[:, hs], in0=t[:, hs], scalar=gam_sb,
                                in1=pose_t[ch][:, hs],
                                op0=ALU.mult, op1=ALU.add)
                            eng = nc.sync if h == 0 else nc.scalar
                            eng.dma_start(out_d[:, ihs], o[:, hs])

    nc.compile()
    return nc


def _get_nc():
    if "nc" not in _CACHE:
        _CACHE["nc"] = _build()
    return _CACHE["nc"]


def kernel(pose_f, id_f, Wq, bq, Wk, bk, Wv, bv, gamma, **run_kwargs):
    pose_f = np.asarray(pose_f, dtype=np.float32)
    id_f = np.asarray(id_f, dtype=np.float32)
    Wq = np.asarray(Wq, dtype=np.float32)
    Wk = np.asarray(Wk, dtype=np.float32)
    Wv = np.asarray(Wv, dtype=np.float32)
    bq = np.asarray(bq, dtype=np.float32)
    bk = np.asarray(bk, dtype=np.float32)
    bv = np.asarray(bv, dtype=np.float32)
    g = float(np.asarray(gamma, dtype=np.float32).reshape(-1)[0])

    bf = ml_dtypes.bfloat16
    wt = np.concatenate([Wq.T, Wk.T, Wv.T], axis=1).astype(bf)  # [C_in, 3C]
    posebf = pose_f.astype(bf)
    idbf = id_f.astype(bf)
    bq_c = np.ascontiguousarray(bq.reshape(C, 1))
    bk_c = np.ascontiguousarray(bk.reshape(C, 1))
    bfin = np.ascontiguousarray((g * bv).reshape(C, 1).astype(np.float32))
    gam = np.full((C, 1), g, dtype=np.float32)

    in_maps = []
    for b in range(B):
        in_maps.append({
            "pose": pose_f[b],
            "posebf": posebf[b],
            "idbf": idbf[b],
            "wt": wt,
            "bq": bq_c,
            "bk": bk_c,
            "bfin": bfin,
            "gam": gam,
        })

    nc = _get_nc()
    res = run_bass_kernel_spmd(nc, in_maps, core_ids=list(range(B)), **run_kwargs)
    out = np.stack([res.results[b]["out"] for b in range(B)], axis=0)
    if run_kwargs:
        _CACHE["last_result"] = res
    return out
